# revision 3
# baseline (speedup 1.0000x reference)
"""Distributed Trainium2 kernel for a single attention head.

Reference computation (W=32, D=4096):
    k = x @ wk; q = x @ wq; v = x @ wv          # [32, 4096] each
    s = min((q @ k.T) / 256, tri_mask)          # [32, 32], tri = +-1e5
    out = softmax(s, axis=1) @ v                # [32, 4096]

Strategy: tensor-parallel over 8 NeuronCores. Core c owns columns
[512c, 512c+512) of wk/wq/wv. It computes its k/q/v slices [32, 512]
(fp16 operands, f32 PSUM), a partial score matrix s_c = q_c @ k_c.T
[32, 32], AllGathers the partial scores (4 KB per rank), sums them
locally, runs the softmax replicated, and produces
out[:, 512c:512c+512] = a @ v_c. The host concatenates the 8 slices.

A dummy warm-up AllGather issues at NEFF start so the cold collective
costs (global entry barrier + ncfw wakeup) overlap the weight DMA
phase instead of sitting on the critical path.

fp16 input quantization was validated against the reference seed:
rel err ~3e-4, min top-2 score margin 6.3 (softmax is near-one-hot).
"""

import numpy as np

N_CORES = 8
W = 32            # window (rows of x)
D = 4096          # in_size
NSH = 512         # output columns per core
CH = D // 128     # 32 d-chunks of 128 rows
GRP = 8           # d-chunks per DMA group (1 MB fp16 per group)
NGRP = CH // GRP  # 4 groups per weight
SCALE = 1.0 / 256.0
MASK_MAG = 100000.0

_CACHE = {}


def _build(warmup=True):
    import sys
    if "/opt/trn_rl_repo" not in sys.path:
        sys.path.insert(0, "/opt/trn_rl_repo")
    import concourse.bass as bass
    import concourse.mybir as mybir
    import concourse.tile as tile
    from concourse import bacc

    f16 = mybir.dt.float16
    f32 = mybir.dt.float32
    rg = [list(range(N_CORES))]

    nc = bacc.Bacc(
        "TRN2",
        target_bir_lowering=False,
        debug=False,
        num_devices=N_CORES,
    )

    # xt[p, c, i] = x[i, 128c + p] (pre-transposed on host, fp16)
    xt_ext = nc.dram_tensor("xt", [128, CH, W], f16, kind="ExternalInput")
    # w*[p, g, s, n] = w[128*(GRP*g+s) + p, 512*core + n] — per-partition
    # contiguous 8 KB per group DMA for full descriptor efficiency.
    wk_ext = nc.dram_tensor("wk", [128, NGRP, GRP, NSH], f16, kind="ExternalInput")
    wq_ext = nc.dram_tensor("wq", [128, NGRP, GRP, NSH], f16, kind="ExternalInput")
    wv_ext = nc.dram_tensor("wv", [128, NGRP, GRP, NSH], f16, kind="ExternalInput")
    mask_ext = nc.dram_tensor("mask", [W, W], f32, kind="ExternalInput")
    out_ext = nc.dram_tensor("out", [W, NSH], f32, kind="ExternalOutput")

    with tile.TileContext(nc) as tc:
        with tc.tile_pool(name="weights", bufs=12) as wpool, \
             tc.tile_pool(name="small", bufs=1) as small, \
             tc.tile_pool(name="psum", bufs=1, space="PSUM") as psum, \
             tc.tile_pool(name="dram", bufs=1, space="DRAM") as dram:

            # ---- warm-up collective: absorb barrier + ncfw wakeup ----
            if warmup:
                wu_sb = small.tile([W, 8], f32, tag="wu_sb")
                nc.vector.memset(wu_sb[:], 0.0)
                wu_in = dram.tile([W, 8], f32, tag="wu_in")
                wu_out = dram.tile([W * N_CORES, 8], f32, tag="wu_out",
                                   addr_space="Shared")
                nc.scalar.dma_start(out=wu_in[:], in_=wu_sb[:])
                nc.gpsimd.collective_compute(
                    "AllGather",
                    mybir.AluOpType.bypass,
                    replica_groups=rg,
                    ins=[wu_in.opt()],
                    outs=[wu_out.opt()],
                )

            # ---- loads ----
            xt_sb = small.tile([128, CH, W], f16, tag="xt")
            nc.sync.dma_start(out=xt_sb[:], in_=xt_ext[:])
            mask_sb = small.tile([W, W], f32, tag="mask")
            nc.sync.dma_start(out=mask_sb[:], in_=mask_ext[:])

            # interleave wk/wq groups so k and q finish together; wv last
            wtiles = {}
            order = []
            for g in range(NGRP):
                order += [("wk", wk_ext, g), ("wq", wq_ext, g)]
            for g in range(NGRP):
                order += [("wv", wv_ext, g)]
            for name, ext, g in order:
                t = wpool.tile([128, GRP, NSH], f16, tag="w")
                nc.sync.dma_start(out=t[:], in_=ext[:, g])
                wtiles[(name, g)] = t

            # ---- k, q, v = x @ w (contract d on partitions) ----
            kps = psum.tile([W, NSH], f32, tag="kps")
            qps = psum.tile([W, NSH], f32, tag="qps")
            vps = psum.tile([W, NSH], f32, tag="vps")

            def group_matmuls(ps, name, g):
                for s in range(GRP):
                    c = g * GRP + s
                    nc.tensor.matmul(
                        ps[:],
                        xt_sb[:, c, :],
                        wtiles[(name, g)][:, s, :],
                        start=(c == 0),
                        stop=(c == CH - 1),
                    )

            for g in range(NGRP):
                group_matmuls(kps, "wk", g)
                group_matmuls(qps, "wq", g)

            # ---- partial scores s_c = q_c @ k_c.T ----
            k_sb = small.tile([W, NSH], f32, tag="k_sb")
            q_sb = small.tile([W, NSH], f32, tag="q_sb")
            nc.vector.tensor_copy(out=k_sb[:], in_=kps[:])
            nc.vector.tensor_copy(out=q_sb[:], in_=qps[:])
            # 32x32-block stream transpose: block b holds kT[32b:32b+32, :]
            kT = small.tile([W, NSH], f32, tag="kT")
            qT = small.tile([W, NSH], f32, tag="qT")
            nc.vector.transpose(kT[:], k_sb[:])
            nc.vector.transpose(qT[:], q_sb[:])

            sps = psum.tile([W, W], f32, tag="sps")
            nb = NSH // W  # 16 blocks of 32 local columns
            for b in range(nb):
                nc.tensor.matmul(
                    sps[:],
                    qT[:, b * W:(b + 1) * W],
                    kT[:, b * W:(b + 1) * W],
                    start=(b == 0),
                    stop=(b == nb - 1),
                )
            s_sb = small.tile([W, W], f32, tag="s_sb")
            nc.vector.tensor_copy(out=s_sb[:], in_=sps[:])

            # ---- v matmuls (after score path so PE frees scores early) ----
            for g in range(NGRP):
                group_matmuls(vps, "wv", g)
            v_sb = small.tile([W, NSH], f32, tag="v_sb")
            nc.vector.tensor_copy(out=v_sb[:], in_=vps[:])

            # ---- AllGather partial scores (4 KB/rank), sum locally ----
            # bounce DMAs ride the ACT HWDGE ring, not the SP ring that is
            # busy draining the weight streams.
            cc_in = dram.tile([W, W], f32, tag="cc_in")
            cc_out = dram.tile([W * N_CORES, W], f32, tag="cc_out",
                               addr_space="Shared")
            nc.scalar.dma_start(out=cc_in[:], in_=s_sb[:])
            nc.gpsimd.collective_compute(
                "AllGather",
                mybir.AluOpType.bypass,
                replica_groups=rg,
                ins=[cc_in.opt()],
                outs=[cc_out.opt()],
            )
            g_sb = small.tile([W, N_CORES, W], f32, tag="g_sb")
            nc.scalar.dma_start(
                out=g_sb[:], in_=cc_out[:].rearrange("(r p) j -> p r j", p=W)
            )
            s_all = small.tile([W, W], f32, tag="s_all")
            nc.vector.tensor_reduce(
                out=s_all[:], in_=g_sb[:].rearrange("p r j -> p j r"),
                axis=mybir.AxisListType.X, op=mybir.AluOpType.add,
            )

            # ---- softmax(min(s/256, mask)) ----
            smin = small.tile([W, W], f32, tag="smin")
            nc.scalar.mul(out=smin[:], in_=s_all[:], mul=SCALE)
            nc.vector.tensor_tensor(
                out=smin[:], in0=smin[:], in1=mask_sb[:], op=mybir.AluOpType.min
            )
            nmax = small.tile([W, 1], f32, tag="nmax")
            nc.vector.tensor_reduce(
                out=nmax[:], in_=smin[:], axis=mybir.AxisListType.X,
                op=mybir.AluOpType.max, negate=True,
            )
            p_sb = small.tile([W, W], f32, tag="p_sb")
            rsum = small.tile([W, 1], f32, tag="rsum")
            nc.scalar.activation(
                out=p_sb[:], in_=smin[:],
                func=mybir.ActivationFunctionType.Exp,
                bias=nmax[:], scale=1.0, accum_out=rsum[:],
            )
            rinv = small.tile([W, 1], f32, tag="rinv")
            nc.vector.reciprocal(rinv[:], rsum[:])
            pT = small.tile([W, W], f32, tag="pT")
            nc.vector.transpose(pT[:], p_sb[:])

            # ---- out = (p @ v) * rinv ----
            ops = psum.tile([W, NSH], f32, tag="ops")
            nc.tensor.matmul(ops[:], pT[:], v_sb[:], start=True, stop=True)
            out_sb = small.tile([W, NSH], f32, tag="out_sb")
            nc.vector.tensor_scalar_mul(out=out_sb[:], in0=ops[:], scalar1=rinv[:])
            nc.scalar.dma_start(out=out_ext[:], in_=out_sb[:])

    nc.compile()
    return nc


def _get_nc():
    if "nc" not in _CACHE:
        _CACHE["nc"] = _build()
    return _CACHE["nc"]


def _w_layout(w, c):
    # [4096, 512] slice -> [128, NGRP, GRP, NSH] with w[128*(GRP*g+s)+p, n]
    # at [p, g, s, n]; per-partition 8 KB contiguous runs per group.
    ws = w[:, c * NSH:(c + 1) * NSH].astype(np.float16)
    return np.ascontiguousarray(
        ws.reshape(NGRP, GRP, 128, NSH).transpose(2, 0, 1, 3)
    )


def _make_in_maps(x, wk, wq, wv):
    xt = np.ascontiguousarray(
        x.T.reshape(CH, 128, W).transpose(1, 0, 2)
    ).astype(np.float16)
    lower = np.tril(np.ones((W, W), dtype=bool))
    mask = np.where(lower, MASK_MAG, -MASK_MAG).astype(np.float32)
    in_maps = []
    for c in range(N_CORES):
        in_maps.append({
            "xt": xt,
            "wk": _w_layout(wk, c),
            "wq": _w_layout(wq, c),
            "wv": _w_layout(wv, c),
            "mask": mask,
        })
    return in_maps


def kernel(x, wk, wq, wv, _trace=False, _trace_kwargs=None):
    import sys
    if "/opt/trn_rl_repo" not in sys.path:
        sys.path.insert(0, "/opt/trn_rl_repo")
    from concourse.bass_utils import run_bass_kernel_spmd

    nc = _get_nc()
    in_maps = _make_in_maps(
        np.asarray(x, dtype=np.float32),
        np.asarray(wk, dtype=np.float32),
        np.asarray(wq, dtype=np.float32),
        np.asarray(wv, dtype=np.float32),
    )
    res = run_bass_kernel_spmd(
        nc, in_maps, core_ids=list(range(N_CORES)),
        trace=_trace, **(_trace_kwargs or {}),
    )
    out = np.concatenate(
        [res.results[c]["out"] for c in range(N_CORES)], axis=1
    ).astype(np.float32)
    if _trace:
        _CACHE["last_result"] = res
    return out
